# revision 2
# baseline (speedup 1.0000x reference)
"""Mamba chunk-state kernel for Trainium2 (8 NeuronCores, Bass/Tile) — v2.

Computes, for inputs
    B  (b=4, s=8192, g=1, n=128)   f32
    x  (b=4, s=8192, h=32, p=64)   f32
    dt (b=4, h=32, c=32, l=256)    f32
    dA (b=4, h=32, c=32, l=256)    f32
the chunked state update
    states[b,c,h,p,n] = sum_l x[b,c,l,h,p] * scale[b,h,c,l] * B[b,c,l,n]
    scale = exp(dA[...,-1:] - dA) * dt

Sharding: core i handles batch b = i//2 and chunk range (i%2)*16..+16.
Each (b, chunk-range) slice is fully independent -> no collectives.

v2 design (vs the fp32 baseline at 113us):
  - x, B are uploaded in bf16; matmuls run in bf16 (1 cyc/row on PE vs 4
    for fp32), output stored as bf16 and upcast on host. Halves DMA
    traffic and quarters PE time. Measured precision (global-absmax rel
    err) 5.3e-3, under the 2e-2 gate.
  - B-stationary matmuls: out[n, hp-block] = sum_l B[l,n]^T xw[l,hp],
    8 matmuls/chunk with 512-wide outputs instead of 32 with 128-wide.
    Output layout [n, hp] makes the store DMA fully contiguous; the host
    transposes back during unshard.
  - dt/dA are uploaded pre-transposed to [l, h] (host-side layout move)
    interleaved in one meta tensor; da_last additionally pre-broadcast
    to all 128 partitions. Scale = exp(da_last - da) * dt is computed on
    device in [l, (half,h)] layout with 4 tiny scalar_tensor_tensor ops
    + one ACT exp: no on-device transposes, no PSUM contention.
  - xw = x * scale is ONE wide scalar_tensor_tensor per chunk ([128,
    (2,32,64)] with a stride-0 broadcast AP on the scale operand), which
    keeps DVE's 2x SBUF perf mode instead of 64 narrow per-head ops.
  - DMA issues spread over SP (x, out) and Pool (B, meta) queues.
"""

import numpy as np
import ml_dtypes

BATCH, SEQLEN, NGROUPS, DSTATE = 4, 8192, 1, 128
NHEADS, HEADDIM, CHUNK = 32, 64, 256
NCHUNKS = SEQLEN // CHUNK  # 32
NCORES = 8
CPC = (BATCH * NCHUNKS) // NCORES  # 16 chunks per core
HP = NHEADS * HEADDIM  # 2048
L = CPC * CHUNK  # 4096 rows per core

BF16 = ml_dtypes.bfloat16

_cached_nc = None


def _build_nc(repeat=1):
    import concourse.bacc as bacc
    import concourse.mybir as mybir
    import concourse.tile as tile

    f32 = mybir.dt.float32
    bf16 = mybir.dt.bfloat16
    Exp = mybir.ActivationFunctionType.Exp
    MULT = mybir.AluOpType.mult
    ADD = mybir.AluOpType.add

    nc = bacc.Bacc(
        "TRN2",
        target_bir_lowering=False,
        debug=False,
        num_devices=NCORES,
    )

    x_d = nc.dram_tensor("x_s", [L, HP], bf16, kind="ExternalInput").ap()
    b_d = nc.dram_tensor("b_s", [L, DSTATE], bf16, kind="ExternalInput").ap()
    # meta: cols 0:32 = dA (transposed to [l, h]), 32:64 = dt
    m_d = nc.dram_tensor("m_s", [L, 64], f32, kind="ExternalInput").ap()
    # da_last pre-broadcast to 128 partitions; col cc*32+h
    dal_d = nc.dram_tensor("dal_s", [128, CPC * 32], f32, kind="ExternalInput").ap()
    out_d = nc.dram_tensor(
        "out_s", [CPC, DSTATE, HP], bf16, kind="ExternalOutput"
    ).ap()

    with tile.TileContext(nc) as tc:
        with (
            tc.tile_pool(name="meta", bufs=1) as meta_pool,
            tc.tile_pool(name="xin", bufs=4) as x_pool,
            tc.tile_pool(name="bin", bufs=3) as b_pool,
            tc.tile_pool(name="min", bufs=3) as m_pool,
            tc.tile_pool(name="scp", bufs=3) as sc_pool,
            tc.tile_pool(name="xwp", bufs=3) as xw_pool,
            tc.tile_pool(name="stgp", bufs=3) as stg_pool,
            tc.tile_pool(name="pstates", bufs=2, space="PSUM") as ps_pool,
        ):
            dal = meta_pool.tile([128, CPC * 32], f32)
            nc.sync.dma_start(dal[:], dal_d[:])

            for cc_rep in range(CPC * repeat):
                cc = cc_rep % CPC
                r0 = cc * CHUNK
                # ---- loads: l-half pairs folded into one tile ----
                # xh[l, (a, h, p)]: cols 0:2048 = l-half 0 rows, 2048:4096 = half 1
                xh = x_pool.tile([128, 2 * HP], bf16, name="xh", tag="xh")
                nc.sync.dma_start(
                    xh[:].rearrange("p (a c) -> p a c", a=2),
                    x_d[r0 : r0 + 256, :].rearrange("(a p) c -> p a c", a=2),
                )
                # bh[l, (a, n)]
                bh = b_pool.tile([128, 2 * DSTATE], bf16, name="bh", tag="bh")
                nc.gpsimd.dma_start(
                    bh[:].rearrange("p (a n) -> p a n", a=2),
                    b_d[r0 : r0 + 256, :].rearrange("(a p) n -> p a n", a=2),
                )
                # mt[l, (a, [da|dt])]: cols a*64+0:32 = da, a*64+32:64 = dt
                mt = m_pool.tile([128, 128], f32, name="mt", tag="mt")
                nc.gpsimd.dma_start(
                    mt[:].rearrange("p (a c) -> p a c", a=2),
                    m_d[r0 : r0 + 256, :].rearrange("(a p) c -> p a c", a=2),
                )

                # ---- scale = exp(da_last - da) * dt in [l, (a, h)] ----
                dal_ap = dal[:, cc * 32 : (cc + 1) * 32]
                tmp = sc_pool.tile([128, 64], f32, name="tmp", tag="tmp")
                dec = sc_pool.tile([128, 64], f32, name="dec", tag="dec")
                scl = sc_pool.tile([128, 64], f32, name="scl", tag="scl")
                for a in range(2):
                    # tmp = -da + da_last
                    nc.vector.scalar_tensor_tensor(
                        tmp[:, a * 32 : (a + 1) * 32],
                        mt[:, a * 64 : a * 64 + 32],
                        -1.0,
                        dal_ap,
                        op0=MULT,
                        op1=ADD,
                    )
                nc.scalar.activation(dec[:], tmp[:], Exp)
                for a in range(2):
                    # scl = dec * dt
                    nc.vector.scalar_tensor_tensor(
                        scl[:, a * 32 : (a + 1) * 32],
                        dec[:, a * 32 : (a + 1) * 32],
                        1.0,
                        mt[:, a * 64 + 32 : a * 64 + 64],
                        op0=MULT,
                        op1=MULT,
                    )

                # ---- xw = x * scale: one wide op, scale broadcast over p ----
                xw = xw_pool.tile([128, 2 * HP], bf16, name="xw", tag="xw")
                scl_bc = (
                    scl[:]
                    .rearrange("p (a h) -> p a h", a=2)
                    .unsqueeze(-1)
                    .broadcast_to([128, 2, NHEADS, HEADDIM])
                )
                nc.vector.scalar_tensor_tensor(
                    xw[:].rearrange("p (a h q) -> p a h q", a=2, h=NHEADS),
                    xh[:].rearrange("p (a h q) -> p a h q", a=2, h=NHEADS),
                    1.0,
                    scl_bc,
                    op0=MULT,
                    op1=MULT,
                )

                # ---- states: out[n, hp] = sum_l B[l,n]^T xw[l,hp] ----
                st = ps_pool.tile([128, HP], f32, name="st", tag="st")
                for a in range(2):
                    w = bh[:, a * DSTATE : (a + 1) * DSTATE]
                    for q in range(4):
                        nc.tensor.matmul(
                            st[:, q * 512 : (q + 1) * 512],
                            w,
                            xw[:, a * HP + q * 512 : a * HP + (q + 1) * 512],
                            start=(a == 0),
                            stop=(a == 1),
                        )
                stg = stg_pool.tile([128, HP], bf16, name="stg", tag="stg")
                nc.scalar.copy(stg[:], st[:])
                nc.sync.dma_start(out_d[cc], stg[:])

    nc.compile()
    return nc


def _get_nc():
    global _cached_nc
    if _cached_nc is None:
        _cached_nc = _build_nc()
    return _cached_nc


def _in_maps(B, x, dt, dA_cumsum):
    B = np.asarray(B, dtype=np.float32)
    x = np.asarray(x, dtype=np.float32)
    dt = np.asarray(dt, dtype=np.float32)
    dA = np.asarray(dA_cumsum, dtype=np.float32)
    maps = []
    for core in range(NCORES):
        b = core // 2
        c0 = (core % 2) * CPC
        s0, s1 = c0 * CHUNK, (c0 + CPC) * CHUNK
        # [l, h] transposed meta: row cc*256+l, col h
        da_tr = (
            dA[b, :, c0 : c0 + CPC, :].transpose(1, 2, 0).reshape(L, NHEADS)
        )
        dt_tr = (
            dt[b, :, c0 : c0 + CPC, :].transpose(1, 2, 0).reshape(L, NHEADS)
        )
        m = np.concatenate([da_tr, dt_tr], axis=1).astype(np.float32)
        dal = (
            dA[b, :, c0 : c0 + CPC, CHUNK - 1].T.reshape(CPC * 32).astype(np.float32)
        )
        maps.append(
            {
                "x_s": np.ascontiguousarray(
                    x[b, s0:s1].reshape(L, HP)
                ).astype(BF16),
                "b_s": np.ascontiguousarray(B[b, s0:s1, 0, :]).astype(BF16),
                "m_s": np.ascontiguousarray(m),
                "dal_s": np.ascontiguousarray(
                    np.broadcast_to(dal, (128, CPC * 32))
                ),
            }
        )
    return maps


def _assemble(results):
    out = np.empty((BATCH, NCHUNKS, NHEADS, HEADDIM, DSTATE), np.float32)
    for core in range(NCORES):
        b = core // 2
        c0 = (core % 2) * CPC
        o = np.asarray(results[core]["out_s"]).astype(np.float32)
        # o[cc] is [n, (h p)] -> [h, p, n]
        out[b, c0 : c0 + CPC] = o.transpose(0, 2, 1).reshape(
            CPC, NHEADS, HEADDIM, DSTATE
        )
    return out


def _run(B, x, dt, dA_cumsum, **run_kwargs):
    from concourse import bass_utils

    nc = _get_nc()
    res = bass_utils.run_bass_kernel_spmd(
        nc, _in_maps(B, x, dt, dA_cumsum), core_ids=list(range(NCORES)), **run_kwargs
    )
    return _assemble(res.results), res


def kernel(B, x, dt, dA_cumsum):
    out, _ = _run(B, x, dt, dA_cumsum)
    return out


# revision 5
# speedup vs baseline: 1.3005x; 1.3005x over previous
"""Mamba chunk-state kernel for Trainium2 (8 NeuronCores, Bass/Tile) — v2.

Computes, for inputs
    B  (b=4, s=8192, g=1, n=128)   f32
    x  (b=4, s=8192, h=32, p=64)   f32
    dt (b=4, h=32, c=32, l=256)    f32
    dA (b=4, h=32, c=32, l=256)    f32
the chunked state update
    states[b,c,h,p,n] = sum_l x[b,c,l,h,p] * scale[b,h,c,l] * B[b,c,l,n]
    scale = exp(dA[...,-1:] - dA) * dt

Sharding: core i handles batch b = i//2 and chunk range (i%2)*16..+16.
Each (b, chunk-range) slice is fully independent -> no collectives.

v2 design (vs the fp32 baseline at 113us):
  - x, B are uploaded in bf16; matmuls run in bf16 (1 cyc/row on PE vs 4
    for fp32), output stored as bf16 and upcast on host. Halves DMA
    traffic and quarters PE time. Measured precision (global-absmax rel
    err) 5.3e-3, under the 2e-2 gate.
  - B-stationary matmuls: out[n, hp-block] = sum_l B[l,n]^T xw[l,hp],
    8 matmuls/chunk with 512-wide outputs instead of 32 with 128-wide.
    Output layout [n, hp] makes the store DMA fully contiguous; the host
    transposes back during unshard.
  - dt/dA are uploaded pre-transposed to [l, h] (host-side layout move)
    interleaved in one meta tensor; da_last additionally pre-broadcast
    to all 128 partitions. Scale = exp(da_last - da) * dt is computed on
    device in [l, (half,h)] layout with 4 tiny scalar_tensor_tensor ops
    + one ACT exp: no on-device transposes, no PSUM contention.
  - xw = x * scale is ONE wide scalar_tensor_tensor per chunk ([128,
    (2,32,64)] with a stride-0 broadcast AP on the scale operand), which
    keeps DVE's 2x SBUF perf mode instead of 64 narrow per-head ops.
  - DMA issues spread over SP (x, out) and Pool (B, meta) queues.
"""

import numpy as np
import ml_dtypes

BATCH, SEQLEN, NGROUPS, DSTATE = 4, 8192, 1, 128
NHEADS, HEADDIM, CHUNK = 32, 64, 256
NCHUNKS = SEQLEN // CHUNK  # 32
NCORES = 8
CPC = (BATCH * NCHUNKS) // NCORES  # 16 chunks per core
HP = NHEADS * HEADDIM  # 2048
L = CPC * CHUNK  # 4096 rows per core

BF16 = ml_dtypes.bfloat16

_cached_nc = None


def _build_nc(repeat=1):
    import concourse.bacc as bacc
    import concourse.mybir as mybir
    import concourse.tile as tile

    f32 = mybir.dt.float32
    bf16 = mybir.dt.bfloat16
    Exp = mybir.ActivationFunctionType.Exp
    MULT = mybir.AluOpType.mult
    ADD = mybir.AluOpType.add

    nc = bacc.Bacc(
        "TRN2",
        target_bir_lowering=False,
        debug=False,
        num_devices=NCORES,
    )

    x_d = nc.dram_tensor("x_s", [L, HP], bf16, kind="ExternalInput").ap()
    b_d = nc.dram_tensor("b_s", [L, DSTATE], bf16, kind="ExternalInput").ap()
    # meta: cols 0:32 = dA (transposed to [l, h]), 32:64 = dt
    m_d = nc.dram_tensor("m_s", [L, 64], f32, kind="ExternalInput").ap()
    # da_last pre-broadcast to 128 partitions; col cc*32+h
    dal_d = nc.dram_tensor("dal_s", [128, CPC * 32], f32, kind="ExternalInput").ap()
    out_d = nc.dram_tensor(
        "out_s", [CPC, DSTATE, HP], bf16, kind="ExternalOutput"
    ).ap()

    with tile.TileContext(nc) as tc:
        with (
            tc.tile_pool(name="meta", bufs=1) as meta_pool,
            tc.tile_pool(name="xin", bufs=5) as x_pool,
            tc.tile_pool(name="bin", bufs=4) as b_pool,
            tc.tile_pool(name="min", bufs=4) as m_pool,
            tc.tile_pool(name="scp", bufs=4) as sc_pool,
            tc.tile_pool(name="xwp", bufs=4) as xw_pool,
            tc.tile_pool(name="stgp", bufs=3) as stg_pool,
            tc.tile_pool(name="pstates", bufs=2, space="PSUM") as ps_pool,
        ):
            dal = meta_pool.tile([128, CPC * 32], f32)
            nc.sync.dma_start(dal[:], dal_d[:])

            for cc_rep in range(CPC * repeat):
                cc = cc_rep % CPC
                r0 = cc * CHUNK
                # ---- loads: l-half pairs folded into one tile ----
                # xh[l, (a, h, p)]: cols 0:2048 = l-half 0 rows, 2048:4096 = half 1
                xh = x_pool.tile([128, 2 * HP], bf16, name="xh", tag="xh")
                nc.sync.dma_start(
                    xh[:].rearrange("p (a c) -> p a c", a=2),
                    x_d[r0 : r0 + 256, :].rearrange("(a p) c -> p a c", a=2),
                )
                # bh[l, (a, n)]
                bh = b_pool.tile([128, 2 * DSTATE], bf16, name="bh", tag="bh")
                nc.sync.dma_start(
                    bh[:].rearrange("p (a n) -> p a n", a=2),
                    b_d[r0 : r0 + 256, :].rearrange("(a p) n -> p a n", a=2),
                )
                # mt[l, (a, [da|dt])]: cols a*64+0:32 = da, a*64+32:64 = dt
                mt = m_pool.tile([128, 128], f32, name="mt", tag="mt")
                nc.sync.dma_start(
                    mt[:].rearrange("p (a c) -> p a c", a=2),
                    m_d[r0 : r0 + 256, :].rearrange("(a p) c -> p a c", a=2),
                )

                # ---- scale = exp(da_last - da) * dt in [l, (a, h)] ----
                dal_ap = dal[:, cc * 32 : (cc + 1) * 32]
                tmp = sc_pool.tile([128, 64], f32, name="tmp", tag="tmp")
                dec = sc_pool.tile([128, 64], f32, name="dec", tag="dec")
                scl = sc_pool.tile([128, 64], f32, name="scl", tag="scl")
                for a in range(2):
                    # tmp = -da + da_last
                    nc.vector.scalar_tensor_tensor(
                        tmp[:, a * 32 : (a + 1) * 32],
                        mt[:, a * 64 : a * 64 + 32],
                        -1.0,
                        dal_ap,
                        op0=MULT,
                        op1=ADD,
                    )
                nc.scalar.activation(dec[:], tmp[:], Exp)
                for a in range(2):
                    # scl = dec * dt
                    nc.vector.scalar_tensor_tensor(
                        scl[:, a * 32 : (a + 1) * 32],
                        dec[:, a * 32 : (a + 1) * 32],
                        1.0,
                        mt[:, a * 64 + 32 : a * 64 + 64],
                        op0=MULT,
                        op1=MULT,
                    )

                # ---- xw = x * scale: per-half wide ops, scale bcast over p ----
                xw = xw_pool.tile([128, 2 * HP], bf16, name="xw", tag="xw")
                for a in range(2):
                    scl_bc = (
                        scl[:, a * 32 : (a + 1) * 32]
                        .unsqueeze(-1)
                        .broadcast_to([128, NHEADS, HEADDIM])
                    )
                    nc.vector.scalar_tensor_tensor(
                        xw[:, a * HP : (a + 1) * HP].rearrange(
                            "p (h q) -> p h q", h=NHEADS
                        ),
                        xh[:, a * HP : (a + 1) * HP].rearrange(
                            "p (h q) -> p h q", h=NHEADS
                        ),
                        1.0,
                        scl_bc,
                        op0=MULT,
                        op1=MULT,
                    )

                # ---- states: out[n, hp] = sum_l B[l,n]^T xw[l,hp] ----
                # s=0 pass starts all 4 banks; s=1 pass stops them, with the
                # PSUM->SBUF cast copy interleaved after banks 1 and 3 so ACT
                # overlaps the tail matmuls.
                st = ps_pool.tile([128, HP], f32, name="st", tag="st")
                stg = stg_pool.tile([128, HP], bf16, name="stg", tag="stg")
                for q in range(4):
                    nc.tensor.matmul(
                        st[:, q * 512 : (q + 1) * 512],
                        bh[:, 0:DSTATE],
                        xw[:, q * 512 : (q + 1) * 512],
                        start=True,
                        stop=False,
                    )
                for q in range(4):
                    nc.tensor.matmul(
                        st[:, q * 512 : (q + 1) * 512],
                        bh[:, DSTATE : 2 * DSTATE],
                        xw[:, HP + q * 512 : HP + (q + 1) * 512],
                        start=False,
                        stop=True,
                    )
                    if q == 1:
                        nc.scalar.copy(stg[:, 0:1024], st[:, 0:1024])
                    elif q == 3:
                        nc.scalar.copy(stg[:, 1024:2048], st[:, 1024:2048])
                nc.gpsimd.dma_start(out_d[cc], stg[:])

    nc.compile()
    return nc


def _get_nc():
    global _cached_nc
    if _cached_nc is None:
        _cached_nc = _build_nc()
    return _cached_nc


def _in_maps(B, x, dt, dA_cumsum):
    B = np.asarray(B, dtype=np.float32)
    x = np.asarray(x, dtype=np.float32)
    dt = np.asarray(dt, dtype=np.float32)
    dA = np.asarray(dA_cumsum, dtype=np.float32)
    maps = []
    for core in range(NCORES):
        b = core // 2
        c0 = (core % 2) * CPC
        s0, s1 = c0 * CHUNK, (c0 + CPC) * CHUNK
        # [l, h] transposed meta: row cc*256+l, col h
        da_tr = (
            dA[b, :, c0 : c0 + CPC, :].transpose(1, 2, 0).reshape(L, NHEADS)
        )
        dt_tr = (
            dt[b, :, c0 : c0 + CPC, :].transpose(1, 2, 0).reshape(L, NHEADS)
        )
        m = np.concatenate([da_tr, dt_tr], axis=1).astype(np.float32)
        dal = (
            dA[b, :, c0 : c0 + CPC, CHUNK - 1].T.reshape(CPC * 32).astype(np.float32)
        )
        maps.append(
            {
                "x_s": np.ascontiguousarray(
                    x[b, s0:s1].reshape(L, HP)
                ).astype(BF16),
                "b_s": np.ascontiguousarray(B[b, s0:s1, 0, :]).astype(BF16),
                "m_s": np.ascontiguousarray(m),
                "dal_s": np.ascontiguousarray(
                    np.broadcast_to(dal, (128, CPC * 32))
                ),
            }
        )
    return maps


def _assemble(results):
    out = np.empty((BATCH, NCHUNKS, NHEADS, HEADDIM, DSTATE), np.float32)
    for core in range(NCORES):
        b = core // 2
        c0 = (core % 2) * CPC
        o = np.asarray(results[core]["out_s"]).astype(np.float32)
        # o[cc] is [n, (h p)] -> [h, p, n]
        out[b, c0 : c0 + CPC] = o.transpose(0, 2, 1).reshape(
            CPC, NHEADS, HEADDIM, DSTATE
        )
    return out


def _run(B, x, dt, dA_cumsum, **run_kwargs):
    from concourse import bass_utils

    nc = _get_nc()
    res = bass_utils.run_bass_kernel_spmd(
        nc, _in_maps(B, x, dt, dA_cumsum), core_ids=list(range(NCORES)), **run_kwargs
    )
    return _assemble(res.results), res


def kernel(B, x, dt, dA_cumsum):
    out, _ = _run(B, x, dt, dA_cumsum)
    return out
